# revision 75
# baseline (speedup 1.0000x reference)
"""Paged prefill attention (sparse_attention) on 8 Trainium2 NeuronCores.

Problem (hardcoded, mirrors the reference):
  q:        [2048, 32, 128] f32   (2 seqs x 1024 query tokens, 32 heads)
  k_cache:  [64, 64, 8, 128] f32  (64 physical blocks x 64 tokens x 8 kv heads)
  v_cache:  [64, 64, 8, 128] f32
  cu_seqlens_q: [0, 1024, 2048]
  cu_seqlens_k: [0, 2048, 4096]
  block_tables: [2, 32] int32 permutation of the 64 physical blocks
  out:      [2048, 32, 128] f32

Sharding: tensor-parallel by kv head. Core h gets kv head h plus its 4
query heads (GQA group 4), both full sequences. One static program runs
SPMD on all 8 cores. Input marshaling (per-core slice, fp16 cast,
[d, token] transposes, block-table ordering of the KV slices) happens on
the host while building each core's input arrays — the device program is
block-table independent.

Per-core device program (S^T layout flash attention, fp16 matmuls),
software-pipelined across all 8 (seq, head) problems:
  - qT [128 d, 8192 (s,h,t)] f16, kT [128 d, 4096 (s,t)] f16 and
    vP [128 tok, 32 chunks x 129] f16 (col 128 of each chunk = ones, the
    softmax denominator accumulator) land via split input DMAs.
  - QK S^T[k,q] per 128-token chunk into a 2-bank PSUM region
    (ping-pong, causal-clipped per chunk).
  - softmax exp: mostly on ScalarE (PSUM -> fp16 es tiles), with the
    small diagonal chunks (12,13) and (14,15) packed pairwise into one
    region/exp call each.  Three history chunks per head ({2,5,7},
    OFFLOAD) instead use a fp16 Schraudolph bit-trick off the ScalarE
    bottleneck: VectorE computes t = score * A + B (f32), GPSIMD
    converts t to int16 whose bits reinterpret as fp16 ~= exp(scale *
    score) within ~3%, with A, B calibrated against the HW
    round-to-nearest conversion.  This takes ScalarE from pacing the
    whole pipeline (90.7us busy) to 82us, under the PE's 86us, at
    rel err 1.62e-2 (vs the 2e-2 gate; verified bit-stable on HW).
  - diagonal chunks: strictly-below-diagonal es zeroed by GPSIMD
    affine_select (off the PE/ACT critical chain).
  - PV accumulates es.T @ vP into PSUM-resident [128, 129]-per-qt
    slots in two banks via three waves per head (qt {0,1,2} and {3,4,5}
    concurrently, {6,7} reusing the first bank after its drain), one
    accumulation group per bank.  PV lags QK by LAG chunks globally
    (cross-head pipelining), leaving six PSUM banks for three QK
    regions so the PE runs far enough ahead to hide the offload gaps.
  - Drain per bank group: one DVE tensor_scalar divide straight from
    PSUM (unnormalized out / ones-column denominator) into the staged
    output, then a per-group DMA out.
"""

import numpy as np

NUM_SEQS = 2
LQ = 1024
HIST = 1024
LK = LQ + HIST
NUM_HEADS = 32
NUM_KV_HEADS = 8
GROUP = NUM_HEADS // NUM_KV_HEADS  # 4 q heads per kv head / core
HEAD_DIM = 128
BLOCK_SIZE = 64
NBLK = LK // BLOCK_SIZE         # 32 logical blocks per sequence
TOTAL_BLOCKS = NUM_SEQS * NBLK  # 64 physical blocks
NCH = LK // 128                 # 16 128-token kv chunks per sequence
NQT = LQ // 128                 # 8 128-token q tiles per sequence
SCALE = 1.0 / float(np.sqrt(HEAD_DIM))

NTOK = NUM_SEQS * LK            # 4096 kv tokens
NQCOL = NUM_SEQS * LQ * GROUP   # 8192 qT columns

import os
LAG = int(os.environ.get("K_LAG", "8"))
K_LAYOUT_LAST = os.environ.get("K_LAYOUT_LAST", "A")
K_LAG_LAST = int(os.environ.get("K_LAG_LAST", "0"))
K_DRIP_LAST = int(os.environ.get("K_DRIP_LAST", "0"))
K_DRIP = int(os.environ.get("K_DRIP", "0"))
# crude (plain Schraudolph, 2-instr) offload chunk sets
OFFLOAD = frozenset(
    int(x) for x in os.environ.get("K_OFF", "2,5,7").split(",") if x)
OFFLOAD2 = frozenset(
    int(x) for x in os.environ.get("K_OFF2", "2,5,7").split(",") if x)
# head 0 fills the cold pipeline: its exp routing can differ
OFFLOAD0 = frozenset(
    int(x) for x in os.environ.get(
        "K_OFF0", "1,2,5,7").split(",") if x)
OFFLOAD_LAST = frozenset(
    int(x) for x in os.environ.get("K_OFF_LAST", "2,5,7").split(",") if x)
# corrected (quadratic mantissa fix, 6-instr) offload chunk sets
OFFLOADC = frozenset(
    int(x) for x in os.environ.get("K_OFFC", "").split(",") if x)
OFFLOADC2 = frozenset(
    int(x) for x in os.environ.get("K_OFFC2", "").split(",") if x)
OFFLOADC_LAST = frozenset(
    int(x) for x in os.environ.get("K_OFFC_LAST", "").split(",") if x)
EXPA = 130.57784916438905       # SCALE * log2(e) * 1024
EXPB = 15360.0                  # 15 * 1024 (no sawtooth centering: the
                                # quadratic mantissa correction handles it)
# minimax quadratic g(m) ~= 2^(m-1)/m on [1,2): es = es_a*(C2 m^2+C1 m+C0)
EXPC2 = 0.21796931
EXPC1 = -0.64039492
EXPC0 = 1.41351975
# which chunks run the u = v*m tensor_tensor of the correction on Pool
# (GPSIMD: slow per-element but otherwise idle) instead of DVE
K_TT_POOL = frozenset(
    int(x) for x in os.environ.get("K_TT_POOL", "2,4,6").split(",") if x)

_CACHE = {}


def _po_slot(qt):
    # po banks hold qt {0,1,2}, {3,4,5}, {6,7}: 129 f32 slots, bank-local
    return (qt // 3) * 512 + (qt % 3) * 129


_DRAIN = {10: (0, 1, 2), 13: (3, 4, 5), 15: (6, 7)}


def _build_program():
    from contextlib import ExitStack

    import concourse.mybir as mybir
    import concourse.tile as tile
    from concourse import bacc

    f32 = mybir.dt.float32
    f16 = mybir.dt.float16
    i16 = mybir.dt.int16

    nc = bacc.Bacc()
    qT_d = nc.dram_tensor("qT", [HEAD_DIM, NQCOL], f16, kind="ExternalInput")
    kT_d = nc.dram_tensor("kT", [HEAD_DIM, NTOK], f16, kind="ExternalInput")
    vP_d = nc.dram_tensor("vP", [128, NUM_SEQS * NCH * 129], f16,
                          kind="ExternalInput")
    o_d = nc.dram_tensor("out", [NUM_SEQS * LQ, GROUP, HEAD_DIM], f32,
                         kind="ExternalOutput")

    with tile.TileContext(nc) as tc, ExitStack() as ctx:
        persist = ctx.enter_context(tc.tile_pool(name="persist", bufs=1))
        import os as _os
        es_pool = ctx.enter_context(tc.tile_pool(name="es", bufs=int(_os.environ.get("K_ESBUFS", "20"))))
        scr_pool = ctx.enter_context(tc.tile_pool(name="scr", bufs=int(_os.environ.get("K_SCRB", "24"))))
        ost_pool = ctx.enter_context(tc.tile_pool(name="ost", bufs=int(_os.environ.get("K_OSTB", "2"))))
        ob_pool = ctx.enter_context(tc.tile_pool(name="ob", bufs=6))
        qk_ps = ctx.enter_context(tc.tile_pool(name="qk_ps", bufs=3,
                                               space="PSUM"))
        po_ps = ctx.enter_context(tc.tile_pool(name="po_ps", bufs=2,
                                               space="PSUM"))

        kT = persist.tile([128, NTOK], f16, tag="kT")
        qT = persist.tile([128, NQCOL], f16, tag="qT")
        vP = persist.tile([128, NUM_SEQS * NCH * 129], f16, tag="vP")

        # split input DMAs, startup-criticality order
        VH = NCH * 129
        dma_order = os.environ.get("K_DMA_ORDER", "W")
        if dma_order == "G":
            pieces = [("k", 0, 128), ("q", 0, 512), ("q", 512, LQ),
                      ("k", 128, 1024), ("v", 0, 1032), ("k", 1024, LK),
                      ("v", 1032, VH), ("q", LQ, 4 * LQ), ("k", LK, NTOK),
                      ("v", VH, 2 * VH), ("q", 4 * LQ, NQCOL)]
        elif dma_order == "D":
            pieces = [("k", 0, 128), ("q", 0, LQ), ("k", 128, 1024),
                      ("v", 0, 1032), ("k", 1024, LK), ("v", 1032, VH),
                      ("q", LQ, 4 * LQ), ("k", LK, NTOK),
                      ("v", VH, 2 * VH), ("q", 4 * LQ, NQCOL)]
        elif dma_order == "E":
            pieces = [("k", 0, 128), ("q", 0, LQ), ("v", 0, 1032),
                      ("k", 128, 1024), ("k", 1024, LK), ("v", 1032, VH),
                      ("q", LQ, 4 * LQ), ("k", LK, NTOK),
                      ("v", VH, 2 * VH), ("q", 4 * LQ, NQCOL)]
        elif dma_order == "H":
            # all of seq-0's k lands before v: the first PV (LAG chunks
            # in) is much later than chunk 8's QK
            pieces = [("k", 0, 128), ("q", 0, LQ), ("k", 128, 1024),
                      ("k", 1024, LK), ("v", 0, 1032), ("v", 1032, VH),
                      ("q", LQ, 4 * LQ), ("k", LK, NTOK),
                      ("v", VH, 2 * VH), ("q", 4 * LQ, NQCOL)]
        elif dma_order == "K":
            pieces = [("k", 0, 128), ("q", 0, 512), ("k", 128, 256),
                      ("q", 512, LQ), ("k", 256, 512), ("k", 512, 1024),
                      ("k", 1024, LK), ("v", 0, 1032), ("v", 1032, VH),
                      ("q", LQ, 4 * LQ), ("k", LK, NTOK),
                      ("v", VH, 2 * VH), ("q", 4 * LQ, NQCOL)]
        elif dma_order == "L":
            pieces = [("k", 0, 128), ("q", 0, 512), ("k", 128, 512),
                      ("q", 512, LQ), ("k", 512, 1024), ("v", 0, 1032),
                      ("k", 1024, LK), ("v", 1032, VH), ("q", LQ, 4 * LQ),
                      ("k", LK, NTOK), ("v", VH, 2 * VH),
                      ("q", 4 * LQ, NQCOL)]
        elif dma_order == "X":
            pieces = [("k", 0, 128), ("q", 0, 512), ("q", 512, LQ),
                      ("k", 128, 256), ("v", 0, 129), ("k", 256, 512),
                      ("k", 512, 1024), ("k", 1024, 1280),
                      ("k", 1280, LK), ("v", 129, 1032), ("v", 1032, VH),
                      ("q", LQ, 4 * LQ), ("k", LK, NTOK),
                      ("v", VH, 2 * VH), ("q", 4 * LQ, NQCOL)]
        elif dma_order == "W":
            # tiny v head: the first PV wave only needs chunk 0's 129
            # v columns, which otherwise land just before the PV fires
            pieces = [("k", 0, 128), ("q", 0, 512), ("q", 512, LQ),
                      ("k", 128, 256), ("k", 256, 512), ("v", 0, 129),
                      ("k", 512, 1024), ("k", 1024, 1280),
                      ("k", 1280, LK), ("v", 129, 1032), ("v", 1032, VH),
                      ("q", LQ, 4 * LQ), ("k", LK, NTOK),
                      ("v", VH, 2 * VH), ("q", 4 * LQ, NQCOL)]
        elif dma_order == "U":
            pieces = [("k", 0, 128), ("q", 0, 512), ("q", 512, LQ),
                      ("k", 128, 256), ("k", 256, 512), ("k", 512, 1024),
                      ("k", 1024, 1280), ("k", 1280, LK), ("v", 0, 1032),
                      ("v", 1032, VH), ("q", LQ, 4 * LQ), ("k", LK, NTOK),
                      ("v", VH, 2 * VH), ("q", 4 * LQ, NQCOL)]
        elif dma_order == "V":
            pieces = [("k", 0, 128), ("q", 0, 512), ("q", 512, LQ),
                      ("k", 128, 256), ("k", 256, 512), ("k", 512, 768),
                      ("k", 768, 1024), ("k", 1024, LK), ("v", 0, 1032),
                      ("v", 1032, VH), ("q", LQ, 4 * LQ), ("k", LK, NTOK),
                      ("v", VH, 2 * VH), ("q", 4 * LQ, NQCOL)]
        elif dma_order == "S":
            # largest first-QK dependency first: the first matmul waits
            # on the later of the k/q semaphores, so q(0:512) leads
            pieces = [("q", 0, 512), ("k", 0, 128), ("q", 512, LQ),
                      ("k", 128, 256), ("k", 256, 512), ("k", 512, 1024),
                      ("k", 1024, LK), ("v", 0, 1032), ("v", 1032, VH),
                      ("q", LQ, 4 * LQ), ("k", LK, NTOK),
                      ("v", VH, 2 * VH), ("q", 4 * LQ, NQCOL)]
        elif dma_order == "T":
            pieces = [("q", 0, 512), ("k", 0, 128), ("k", 128, 256),
                      ("q", 512, LQ), ("k", 256, 512), ("k", 512, 1024),
                      ("k", 1024, LK), ("v", 0, 1032), ("v", 1032, VH),
                      ("q", LQ, 4 * LQ), ("k", LK, NTOK),
                      ("v", VH, 2 * VH), ("q", 4 * LQ, NQCOL)]
        elif dma_order == "Q":
            pieces = [("k", 0, 128), ("q", 0, 512), ("q", 512, LQ),
                      ("k", 128, 256), ("k", 256, 512), ("k", 512, 1024),
                      ("k", 1024, LK), ("v", 0, 1032), ("v", 1032, VH),
                      ("q", LQ, 4 * LQ), ("k", LK, NTOK),
                      ("v", VH, 2 * VH), ("q", 4 * LQ, NQCOL)]
        elif dma_order == "R":
            pieces = [("k", 0, 128), ("q", 0, 512), ("k", 128, 256),
                      ("q", 512, LQ), ("k", 256, 384), ("k", 384, 512),
                      ("k", 512, 1024), ("k", 1024, LK), ("v", 0, 1032),
                      ("v", 1032, VH), ("q", LQ, 4 * LQ), ("k", LK, NTOK),
                      ("v", VH, 2 * VH), ("q", 4 * LQ, NQCOL)]
        elif dma_order == "N":
            pieces = [("k", 0, 128), ("q", 0, 512), ("k", 128, 512),
                      ("q", 512, LQ), ("k", 512, 768), ("k", 768, 1024),
                      ("k", 1024, LK), ("v", 0, 1032), ("v", 1032, VH),
                      ("q", LQ, 4 * LQ), ("k", LK, NTOK),
                      ("v", VH, 2 * VH), ("q", 4 * LQ, NQCOL)]
        elif dma_order == "P":
            pieces = [("k", 0, 128), ("q", 0, 512), ("k", 128, 512),
                      ("k", 512, 1024), ("q", 512, LQ), ("k", 1024, LK),
                      ("v", 0, 1032), ("v", 1032, VH), ("q", LQ, 4 * LQ),
                      ("k", LK, NTOK), ("v", VH, 2 * VH),
                      ("q", 4 * LQ, NQCOL)]
        elif dma_order == "J":
            pieces = [("k", 0, 128), ("q", 0, 512), ("k", 128, 512),
                      ("q", 512, LQ), ("k", 512, 1024), ("k", 1024, LK),
                      ("v", 0, 1032), ("v", 1032, VH), ("q", LQ, 4 * LQ),
                      ("k", LK, NTOK), ("v", VH, 2 * VH),
                      ("q", 4 * LQ, NQCOL)]
        elif dma_order == "I":
            pieces = [("k", 0, 128), ("q", 0, LQ), ("k", 128, 512),
                      ("k", 512, 1024), ("k", 1024, LK), ("v", 0, 1032),
                      ("v", 1032, VH), ("q", LQ, 4 * LQ), ("k", LK, NTOK),
                      ("v", VH, 2 * VH), ("q", 4 * LQ, NQCOL)]
        elif dma_order == "F":
            pieces = [("k", 0, 128), ("q", 0, 2 * LQ), ("k", 128, 1024),
                      ("v", 0, 1032), ("k", 1024, LK), ("v", 1032, VH),
                      ("q", 2 * LQ, 4 * LQ), ("k", LK, NTOK),
                      ("v", VH, 2 * VH), ("q", 4 * LQ, NQCOL)]
        elif dma_order == "A":
            pieces = [("q", 0, LQ), ("k", 0, 128), ("k", 128, 1024),
                      ("v", 0, 1032), ("k", 1024, LK), ("v", 1032, VH),
                      ("q", LQ, 4 * LQ), ("k", LK, NTOK),
                      ("v", VH, 2 * VH), ("q", 4 * LQ, NQCOL)]
        elif dma_order == "B":
            pieces = [("q", 0, LQ), ("k", 0, 1024), ("v", 0, 1032),
                      ("k", 1024, LK), ("v", 1032, VH),
                      ("q", LQ, 4 * LQ), ("k", LK, NTOK),
                      ("v", VH, 2 * VH), ("q", 4 * LQ, NQCOL)]
        else:
            pieces = [("q", 0, LQ), ("kp", 0, 128), ("kp", 128, 1024),
                      ("v", 0, 1032), ("k", 1024, LK), ("v", 1032, VH),
                      ("q", LQ, 4 * LQ), ("k", LK, NTOK),
                      ("v", VH, 2 * VH), ("q", 4 * LQ, NQCOL)]
        srcs = {"q": (qT, qT_d), "k": (kT, kT_d), "v": (vP, vP_d)}
        if os.environ.get("K_DMA_MQ", "0") == "2":
            # first pieces on separate queues: their fixed ~2.9us DMA
            # init latencies overlap, so k+q for the whole first head
            # land together instead of serially
            qengs = [nc.sync, nc.scalar, nc.gpsimd]
            for i, (t, a, b) in enumerate(pieces):
                dst, sd = srcs[t[0]]
                eng = qengs[i] if i < len(qengs) else nc.sync
                eng.dma_start(out=dst[:, a:b], in_=sd[:, a:b])
        elif os.environ.get("K_DMA_MQ", "0") == "1":
            # spread the startup DMAs across engine queues so their
            # fixed init latencies (~2.9us each) overlap instead of
            # serializing on one queue
            qengs = [nc.sync, nc.scalar, nc.gpsimd]
            for i, (t, a, b) in enumerate(pieces):
                dst, sd = srcs[t[0]]
                qengs[i % len(qengs)].dma_start(
                    out=dst[:, a:b], in_=sd[:, a:b])
        else:
            for t, a, b in pieces:
                dst, sd = srcs[t[0]]
                eng = nc.gpsimd if t.endswith("p") else nc.sync
                eng.dma_start(out=dst[:, a:b], in_=sd[:, a:b])

        # PE p-state warmup: the tensor engine ramps to full clock only
        # after ~3us of continuous execution.  The PE is idle during the
        # input-DMA window anyway, so dependency-free matmuls on a
        # zeroed tile carry the ramp so the first real QK runs at speed.
        nwarm = int(os.environ.get("K_WARM", "0"))
        if nwarm:
            wsrc = persist.tile([128, 512], f16, tag="warm")
            nc.gpsimd.memset(wsrc[:, :], 0.0)
            wps = po_ps.tile([128, 512], f32, tag="po", name="warm_ps")
            for _ in range(nwarm):
                nc.tensor.matmul(wps[:, 0:512], wsrc[:, 0:128],
                                 wsrc[:, 0:512], start=True, stop=True)

        heads = [(s, h) for s in range(NUM_SEQS) for h in range(GROUP)]
        state = {}  # hi -> dict(po=, ost=, es=)
        es_ready = set()

        # corrected-Schraudolph chains are 6 dependent DVE instructions;
        # emitted back-to-back they head-of-line-block DVE's in-order
        # queue (ENG_WAIT_QUEUE_DEPTH=4) and stall drains behind them.
        # Instead: the t-pass goes inline (its dep is already satisfied
        # and it frees the QK PSUM region), and the remaining stages are
        # software-pipelined one-per-QK-group so each stage's dependency
        # is a full group old by the time it dispatches.
        schr_chains = {}    # (hi, c) -> [remaining stage closures]
        from collections import OrderedDict as _OrderedDict
        schr_live = _OrderedDict()   # insertion-ordered keys

        def pump_chains():
            for key in list(schr_live):
                stages = schr_chains[key]
                stages.pop(0)()
                if not stages:
                    del schr_chains[key]
                    del schr_live[key]

        def flush_chain(key):
            stages = schr_chains.pop(key, None)
            if stages:
                del schr_live[key]
                for fn in stages:
                    fn()

        def diag_select(hi, c):
            es, base = state[hi]["es"][c]
            nc.gpsimd.affine_select(
                out=es[:, base:base + 128],
                in_=es[:, base:base + 128],
                compare_op=mybir.AluOpType.is_ge, fill=0.0,
                base=0, pattern=[[1, 128]], channel_multiplier=-1)

        def emit_qk_group(hi, group):
            # one PSUM region + one exp call for a group of chunks
            pump_chains()
            s, h = heads[hi]
            qk0set = frozenset(
                int(x) for x in
                os.environ.get("K_QK0", "0,4").split(",") if x != "")
            if (hi == 0 and len(group) == 1 and group[0] in qk0set
                    and group[0] < 8):
                # the po banks are idle until the first PV (LAG groups
                # in): early chunks borrow them as two 512-col regions
                # so the 3-deep qk ring serves other chunks — a deeper
                # pipeline during the cold fill
                c = group[0]
                es = es_pool.tile([128, 1024], f16, tag="es")
                state[0]["es"][c] = (es, 0)
                es_ready.add((0, c))
                for half in range(2):
                    o = half * 512
                    pshalf = po_ps.tile([128, 512], f32, tag="po",
                                        name="qk0h")
                    nc.tensor.matmul(
                        pshalf[:, 0:512],
                        kT[:, c * 128:(c + 1) * 128],
                        qT[:, o:o + 512], start=True, stop=True)
                    nc.scalar.activation(
                        es[:, o:o + 512], pshalf[:, 0:512],
                        mybir.ActivationFunctionType.Exp, scale=SCALE)
                return
            qbase = (s * GROUP + h) * LQ
            ps = qk_ps.tile([128, 1024], f32, tag="qk")
            offs, off = {}, 0
            for c in group:
                offs[c] = off
                off += LQ - max(0, (c - 8) * 128)
            total = off
            # PSUM zeroing is per 2KB bank (512 f32): bracket one
            # accumulation group per bank — start on the bank's first
            # (bank-aligned) piece, stop on its last
            pieces_mm = []
            for gi, c in enumerate(group):
                q_lo = max(0, (c - 8) * 128)
                width = LQ - q_lo
                base = offs[c]
                for mo in range(0, width, 512):
                    n = min(512, width - mo)
                    pieces_mm.append((c, q_lo, base, mo, n))
            by_bank = {}
            for pi, (c, q_lo, base, mo, n) in enumerate(pieces_mm):
                by_bank.setdefault((base + mo) // 512, []).append(pi)
            for b, pis in by_bank.items():
                for pi in pis:
                    c, q_lo, base, mo, n = pieces_mm[pi]
                    nc.tensor.matmul(
                        ps[:, base + mo:base + mo + n],
                        kT[:, s * LK + c * 128:s * LK + (c + 1) * 128],
                        qT[:, qbase + q_lo + mo:qbase + q_lo + mo + n],
                        start=pi == pis[0], stop=pi == pis[-1])
            es = es_pool.tile([128, 1024], f16, tag="es")
            sbase = 0
            for c in group:
                state[hi]["es"][c] = (es, offs[c])
                es_ready.add((hi, c))
            c0 = group[0]
            if hi == 0:
                off_set, offc_set = OFFLOAD0, OFFLOADC
            elif hi == len(heads) - 1:
                off_set, offc_set = OFFLOAD_LAST, OFFLOADC_LAST
            elif hi % 2 == 1:
                off_set, offc_set = OFFLOAD2, OFFLOADC2
            else:
                off_set, offc_set = OFFLOAD, OFFLOADC
            if c0 in off_set:
                assert len(group) == 1
                if os.environ.get("K_CRUDE1", "0") == "1":
                    # single-instruction Schraudolph: DVE's f32->i16
                    # writeback convert is bit-identical to the Pool
                    # convert (HW-verified), so the affine and the
                    # round-to-int fuse into one DVE op
                    nc.vector.tensor_scalar(
                        out=es[:, sbase:sbase + total].bitcast(i16),
                        in0=ps[:, 0:total],
                        scalar1=EXPA, scalar2=15308.0,
                        op0=mybir.AluOpType.mult, op1=mybir.AluOpType.add)
                else:
                    scr = scr_pool.tile([128, 1024], f32, tag="scr")
                    nc.vector.tensor_scalar(
                        out=scr[:, 0:total], in0=ps[:, 0:total],
                        scalar1=EXPA, scalar2=15308.0,
                        op0=mybir.AluOpType.mult, op1=mybir.AluOpType.add)
                    nc.gpsimd.tensor_copy(
                        out=es[:, sbase:sbase + total].bitcast(i16),
                        in_=scr[:, 0:total])
                if c0 >= 8:
                    diag_select(hi, c0)
            elif c0 in offc_set:
                # corrected Schraudolph: t16 = rint(ps*EXPA+EXPB) are the
                # fp16 bits of es_a = 2^floor(x)*(1+f); m = 1+f recovered
                # from the mantissa field; es = es_a * (C2 m^2 + C1 m + C0)
                # corrects the linear-mantissa approximation to ~9e-3.
                assert len(group) == 1
                esa = scr_pool.tile([128, 1024], f16, tag="esa")
                mm = scr_pool.tile([128, 1024], f16, tag="mm")
                vv = scr_pool.tile([128, 1024], f16, tag="vv")
                uu = scr_pool.tile([128, 1024], f16, tag="uu")
                ww = scr_pool.tile([128, 1024], f16, tag="ww")
                nc.vector.tensor_scalar(
                    out=esa[:, 0:total].bitcast(i16), in0=ps[:, 0:total],
                    scalar1=EXPA, scalar2=EXPB,
                    op0=mybir.AluOpType.mult, op1=mybir.AluOpType.add)
                T = total

                def s_m():
                    nc.vector.tensor_scalar(
                        out=mm[:, 0:T].bitcast(i16),
                        in0=esa[:, 0:T].bitcast(i16),
                        scalar1=0x3FF, scalar2=0x3C00,
                        op0=mybir.AluOpType.bitwise_and,
                        op1=mybir.AluOpType.bitwise_or)

                def s_v():
                    nc.vector.tensor_scalar(
                        out=vv[:, 0:T], in0=mm[:, 0:T],
                        scalar1=EXPC2, scalar2=EXPC1,
                        op0=mybir.AluOpType.mult, op1=mybir.AluOpType.add)

                def s_u():
                    eng = nc.gpsimd if c0 in K_TT_POOL else nc.vector
                    eng.tensor_tensor(
                        out=uu[:, 0:T], in0=vv[:, 0:T],
                        in1=mm[:, 0:T], op=mybir.AluOpType.mult)

                def s_w():
                    nc.vector.tensor_scalar(
                        out=ww[:, 0:T], in0=uu[:, 0:T],
                        scalar1=EXPC0, scalar2=None,
                        op0=mybir.AluOpType.add)

                def s_es():
                    nc.vector.tensor_tensor(
                        out=es[:, sbase:sbase + T], in0=ww[:, 0:T],
                        in1=esa[:, 0:T], op=mybir.AluOpType.mult)
                    if c0 >= 8:
                        diag_select(hi, c0)
                key = (hi, c0)
                schr_chains[key] = [s_m, s_v, s_u, s_w, s_es]
                schr_live[key] = True
            else:
                nc.scalar.activation(
                    es[:, sbase:sbase + total], ps[:, 0:total],
                    mybir.ActivationFunctionType.Exp, scale=SCALE)
                for c in group:
                    if c >= 8:
                        diag_select(hi, c)

        def drain(hi, wave, qts, base_qt0=None, dma_eng=None):
            s, h = heads[hi]
            st = state[hi]
            po, ost = st["po"][wave], st["ost"]
            perqt = hi == len(heads) - 1 and wave == 2
            nq = len(qts)
            if base_qt0 is not None:
                # split queued wave: qts is a sub-range of the wave's
                # bank whose slots start at base_qt0
                rc = ob_pool.tile([128, 1], f32, tag="rc", name="rc")
                for qt in qts:
                    sl = (qt - base_qt0) * 129
                    nc.vector.reciprocal(rc[:, :],
                                         po[:, sl + 128:sl + 129])
                    nc.vector.tensor_scalar_mul(
                        ost[:, qt * 128:(qt + 1) * 128],
                        po[:, sl:sl + 128], rc[:, :])
                    r0, r1 = qt * 128, (qt + 1) * 128
                    o_view = o_d[s * LQ + r0:s * LQ + r1, h, :].rearrange(
                        "(c p) d -> p c d", p=128)
                    (dma_eng or nc.sync).dma_start(
                        out=o_view,
                        in_=ost[:, r0:r1].rearrange(
                            "p (c d) -> p c d", d=128))
                return
            if os.environ.get("K_BDRAIN", "0") == "1":
                # batched normalize: one reciprocal + one broadcast
                # multiply per wave instead of per qt
                rc = ob_pool.tile([128, 4], f32, tag="rc", name="rc")
                po3 = po[:, 0:nq * 129].rearrange("p (c n) -> p c n", c=nq)
                nc.vector.reciprocal(
                    rc[:, 0:nq].rearrange("p (c n) -> p c n", n=1),
                    po3[:, :, 128:129])
                nc.vector.tensor_tensor(
                    out=ost[:, qts[0] * 128:(qts[-1] + 1) * 128].rearrange(
                        "p (c n) -> p c n", c=nq),
                    in0=po3[:, :, 0:128],
                    in1=rc[:, 0:nq].rearrange(
                        "p (c n) -> p c n", n=1).broadcast_to(
                            [128, nq, 128]),
                    op=mybir.AluOpType.mult)
            else:
                it_qts = tuple(reversed(qts)) if perqt else qts
                base_qt = min(qts)
                for qt in it_qts:
                    sl = (qt - base_qt) * 129
                    rc = ob_pool.tile([128, 1], f32, tag="rc", name="rc")
                    nc.vector.reciprocal(rc[:, :],
                                         po[:, sl + 128:sl + 129])
                    nc.vector.tensor_scalar_mul(
                        ost[:, qt * 128:(qt + 1) * 128],
                        po[:, sl:sl + 128], rc[:, :])
            if perqt:
                for qt in reversed(qts):
                    r0, r1 = qt * 128, (qt + 1) * 128
                    o_view = o_d[s * LQ + r0:s * LQ + r1, h, :].rearrange(
                        "(c p) d -> p c d", p=128)
                    eng = (nc.gpsimd
                           if os.environ.get("K_TAILQ", "0") == "1"
                           else nc.sync)
                    eng.dma_start(
                        out=o_view,
                        in_=ost[:, r0:r1].rearrange(
                            "p (c d) -> p c d", d=128))
            else:
                r0, r1 = qts[0] * 128, (qts[-1] + 1) * 128
                o_view = o_d[s * LQ + r0:s * LQ + r1, h, :].rearrange(
                    "(c p) d -> p c d", p=128)
                nc.sync.dma_start(
                    out=o_view,
                    in_=ost[:, r0:r1].rearrange("p (c d) -> p c d", d=128))

        def wave_layout(hi):
            # (wave_idx, qts) for the two inline waves, plus the queued
            # wave that reuses wave-0's bank after its c==10 drain.  The
            # last head keeps {6,7} inline so after the final exp only
            # one PV matmul remains before the last drain+DMA.
            if hi == len(heads) - 1 and K_LAYOUT_LAST == "B":
                return [(0, (0, 1, 2)), (1, (6, 7))], (2, (3, 4, 5))
            return [(0, (0, 1, 2)), (1, (3, 4, 5))], (2, (6, 7))

        def pv_mm(st, wave, qts, qt, c, s):
            po = st["po"][wave]
            q_lo = max(0, (c - 8) * 128)
            es, base = st["es"][c]
            sl = (qt - qts[0]) * 129
            col = base + qt * 128 - q_lo
            nc.tensor.matmul(
                po[:, sl:sl + 129],
                es[:, col:col + 128],
                vP[:, (s * NCH + c) * 129:(s * NCH + c + 1) * 129],
                start=(c == 0 and qt == qts[0]),
                stop=(c == qts[-1] + 8 and qt == qts[-1]))

        def emit_pv2(hi, c):
            flush_chain((hi, c))
            s, h = heads[hi]
            st = state[hi]
            _, (wave, qts) = wave_layout(hi)
            if st["po"][wave] is None:
                st["po"][wave] = po_ps.tile([128, 512], f32, tag="po",
                                            name="po2")
            for qt in qts:
                if c - 8 <= qt and c <= qt + 8:
                    pv_mm(st, wave, qts, qt, c, s)
            st["es"].pop(c, None)
            last = hi == len(heads) - 1
            if last and os.environ.get("K_TAILSPLIT", "0") == "1":
                # drain each tail qt as soon as its accumulation closes
                # (qt's last chunk is qt+8) so the final DMA chain is
                # one qt deep instead of the whole wave
                for qt in qts:
                    if c == qt + 8:
                        eng = nc.scalar if qt % 2 else nc.sync
                        drain(hi, wave, (qt,), base_qt0=qts[0],
                              dma_eng=eng)
            elif c == qts[-1] + 8:
                drain(hi, wave, qts)

        def emit_pv(hi, c):
            flush_chain((hi, c))
            s, h = heads[hi]
            st = state[hi]
            inline, (w2, qts2) = wave_layout(hi)
            if c == 0:
                st["po"][inline[0][0]] = po_ps.tile(
                    [128, 512], f32, tag="po", name="po0")
                st["po"][inline[1][0]] = po_ps.tile(
                    [128, 512], f32, tag="po", name="po1")
                st["ost"] = ost_pool.tile([128, LQ], f32, tag="ost",
                                          name="ost")
            for wave, qts in inline:
                for qt in qts:
                    if c - 8 <= qt and c <= qt + 8:
                        pv_mm(st, wave, qts, qt, c, s)
                if c == qts[-1] + 8:
                    drain(hi, wave, qts)
            last = hi == len(heads) - 1
            if last and c == 11:
                for _ in range(int(os.environ.get("K_LASTFLUSH", "0"))):
                    if st["w2q"]:
                        emit_pv2(hi, st["w2q"].popleft())
            if c <= qts2[-1] + 8:
                st["w2q"].append(c)
            else:
                st["es"].pop(c, None)
            if c >= 10:
                if last and c == 10 and os.environ.get("K_BURST10"):
                    while st["w2q"]:
                        emit_pv2(hi, st["w2q"].popleft())
                n = K_DRIP_LAST if last else (K_DRIP if c > 10 else 0)
                for _ in range(n):
                    if st["w2q"]:
                        emit_pv2(hi, st["w2q"].popleft())
            if c == NCH - 1:
                while st["w2q"]:
                    emit_pv2(hi, st["w2q"].popleft())

        from collections import deque
        if os.environ.get("K_GROUPS", "O") == "P":
            # narrow query chunks packed pairwise into full 1024-col
            # exp tiles: 13 groups/head, 11 ACT exps after offload
            groups = [[0], [1], [2], [3], [4], [5], [6], [7], [8],
                      [9, 15], [10, 14], [11, 13], [12]]
        else:
            groups = [[0], [1], [2], [3], [4], [5], [6], [7], [8], [9],
                      [10], [11], [12, 13], [14, 15]]
        qk_ops = [(hi, g) for hi in range(len(heads)) for g in groups]
        chunk_seq = [(hi, c) for hi in range(len(heads)) for c in range(NCH)]

        for hi in range(len(heads)):
            state[hi] = {"po": [None, None, None], "ost": None,
                         "es": {}, "w2q": deque()}
        n_chunks = len(chunk_seq)
        pv_ptr = 0
        done = 0
        for hi, g in qk_ops:
            emit_qk_group(hi, g)
            done += len(g)
            if K_LAG_LAST == 0:
                lag = max(1, min(LAG, n_chunks - done))
            else:
                lag = K_LAG_LAST if hi == len(heads) - 1 else LAG
            ramp = int(os.environ.get("K_LAGRAMP", "0"))
            if ramp:
                # small lag during the first head's pipeline fill: PE
                # gets PV work while ACT is still ramping the es stream
                lag = min(lag, max(ramp, done - 8))
            lag0 = int(os.environ.get("K_LAG0", "0"))
            if lag0 and hi == 0:
                lag = lag0
            while (pv_ptr <= done - 1 - lag
                   and chunk_seq[pv_ptr] in es_ready):
                emit_pv(*chunk_seq[pv_ptr])
                pv_ptr += 1
        while pv_ptr < n_chunks:
            emit_pv(*chunk_seq[pv_ptr])
            pv_ptr += 1
        for key in list(schr_live):
            flush_chain(key)

    nc.compile()
    return nc


def _get_program():
    if "prog" not in _CACHE:
        _CACHE["prog"] = _build_program()
    return _CACHE["prog"]


def _marshal_core(q, k_cache, v_cache, rows, core):
    """Build one core's input arrays: fp16, transposed, block-table order."""
    q16 = np.ascontiguousarray(
        q[:, core * GROUP:(core + 1) * GROUP, :]).astype(np.float16)
    # qT[d, s*4096 + h*1024 + t] = q[s*1024 + t, h, d]
    qT = np.ascontiguousarray(
        q16.reshape(NUM_SEQS, LQ, GROUP, HEAD_DIM)
        .transpose(3, 0, 2, 1).reshape(HEAD_DIM, NQCOL))

    k16 = k_cache[:, :, core, :].reshape(NTOK, HEAD_DIM).astype(np.float16)
    v16 = v_cache[:, :, core, :].reshape(NTOK, HEAD_DIM).astype(np.float16)
    kT = np.ascontiguousarray(k16[rows].T)           # [128, 4096]

    vl = v16[rows].reshape(NUM_SEQS * NCH, 128, HEAD_DIM)
    vP = np.ones((128, NUM_SEQS * NCH, 129), dtype=np.float16)
    vP[:, :, 0:HEAD_DIM] = vl.transpose(1, 0, 2)
    return {"qT": qT, "kT": kT,
            "vP": np.ascontiguousarray(vP.reshape(128, NUM_SEQS * NCH * 129))}


def kernel(q, k_cache, v_cache, cu_seqlens_q, cu_seqlens_k, block_tables,
           _want_trace=False):
    from concourse import bass_utils

    q = np.asarray(q, dtype=np.float32)
    k_cache = np.asarray(k_cache, dtype=np.float32)
    v_cache = np.asarray(v_cache, dtype=np.float32)
    bt = np.asarray(block_tables, dtype=np.int32)

    assert q.shape == (NUM_SEQS * LQ, NUM_HEADS, HEAD_DIM)
    assert k_cache.shape == (TOTAL_BLOCKS, BLOCK_SIZE, NUM_KV_HEADS, HEAD_DIM)
    assert v_cache.shape == (TOTAL_BLOCKS, BLOCK_SIZE, NUM_KV_HEADS, HEAD_DIM)
    assert bt.shape == (NUM_SEQS, NBLK)
    assert bt.min() >= 0

    nc = _get_program()

    # DRAM row of logical kv token (s, t): block-table gather order
    t = np.arange(LK, dtype=np.int64)
    rows = np.concatenate(
        [bt[s, t // BLOCK_SIZE] * BLOCK_SIZE + t % BLOCK_SIZE
         for s in range(NUM_SEQS)])

    in_maps = [_marshal_core(q, k_cache, v_cache, rows, core)
               for core in range(NUM_KV_HEADS)]

    res = bass_utils.run_bass_kernel_spmd(
        nc, in_maps, core_ids=list(range(NUM_KV_HEADS)),
        trace=_want_trace,
        **({"trace_cores": list(range(NUM_KV_HEADS)), "stitch_traces": True}
           if _want_trace else {}),
    )

    out = np.empty((NUM_SEQS * LQ, NUM_HEADS, HEAD_DIM), dtype=np.float32)
    for core in range(NUM_KV_HEADS):
        out[:, core * GROUP:(core + 1) * GROUP, :] = res.results[core]["out"]

    if _want_trace:
        return out, res
    return out

